# revision 11
# baseline (speedup 1.0000x reference)
"""Trainium2 Bass kernel for the Kagome-lattice masked directional CNN.

Strategy (pure data-parallel over batch, 8 cores, 256 samples each):
  - Host pre-computes the padded 18x18 image xp (zero pad + 30 periodic
    boundary copies), drops the all-zero row 0 and column 17, and
    de-interleaves columns into [9 even | 8 odd] per row.  Each image is
    17x17 = 289 fp16 elements; every conv tap then reads a contiguous
    stride-1 run of 8 values per output row.
  - Images are packed 8 per "unit"; units 0-15 live in SBUF partitions
    0-63 (channel = partition), units 16-31 in partitions 64-127.  The
    whole per-core input (74 KB/partition) is resident in SBUF, loaded
    with long per-partition contiguous DMA runs (18.5 KB).
  - Each directional conv decomposes into 5 per-tap matmuls (K=64=c).
    A "tile" = 16 images = one unit in each partition half; every tap
    issues as two K=64 matmuls on PE row groups (0/64) which run
    concurrently.  L/R-only taps pair across column groups, and the U
    taps of a pair of tiles share one PSUM tile (even tile -> psum
    partitions 0-63, odd -> 64-127) so they run 4-way concurrent.  No
    correction matmuls: the ring cells are present in the padded layout.
  - PSUM accumulates [o, img*64 + p*8 + q] per half; Scalar engine
    copies LR psum -> fp16 SBUF, Vector engine copies U psum.  Output
    is the 3 live sub-lattices only (fp16); host re-interleaves into
    the 16x16 grid, adds biases, and applies the static output mask.
"""

import sys
import functools

import numpy as np

if "/opt/trn_rl_repo" not in sys.path:
    sys.path.insert(0, "/opt/trn_rl_repo")

# ---------------------------------------------------------------- constants
B, C, O = 2048, 64, 64
NCORES = 8
BC = B // NCORES           # samples per core
IMG = 289                  # 17x17 de-interleaved padded image
UNIT = 8 * IMG             # 8 images per unit (2312 elems)
NUNITS = BC // 8           # 32 units -> 16 tiles x 2 partition halves
NTILES = NUNITS // 2       # 16
NGROUPS = 4                # tiles per out/in DMA batch
TPG = NTILES // NGROUPS    # 4 tiles per group

DST_R = np.array([1,1,2,3,4,4,6,7,8,10,11,12,14,14,15,16,17,17,16,15,14,14,12,10,8,6,4,4,3,2])
DST_C = np.array([3,5,7,9,10,11,13,13,14,15,15,16,15,16,15,14,13,11,9,7,6,5,3,2,1,0,0,1,1,2])
SRC_R = np.array([13,13,14,15,16,16,6,7,8,10,11,12,2,2,3,4,5,5,4,3,2,2,12,10,8,6,16,16,15,14])
SRC_C = np.array([15,5,7,9,10,11,1,1,2,3,3,4,3,4,3,2,1,11,9,7,6,5,15,14,13,12,12,13,13,14])


def _out_mask():
    m = np.ones((16, 16), np.float32)
    for i in range(9):
        m[i, 7 + i:16] = 0
    for i in range(7):
        m[9 + i, 0:i + 1] = 0
    m[0,4:7]=0; m[1,6:8]=0; m[2,8]=0; m[3,9]=0
    m[6,12]=0; m[7,13]=0; m[8,14]=0; m[9,14]=0; m[10,14]=0; m[11,15]=0
    m[13:,14:]=0; m[15,13]=0; m[15,7:9]=0; m[13,5]=0; m[14,6]=0
    m[8,0]=0; m[9,1]=0; m[7,0]=0; m[3,0]=0; m[0:3,0:2]=0; m[0,2]=0
    return m


OUT_MASK = _out_mask()

# De-interleaved tap offset within an image: rows are xp rows 1..17,
# 17 elements each ([even xp cols 0,2..16 | odd xp cols 1,3..15]).
# Output (p,q) of tap (dr,dc) reads element off + 34*p + q.
def _tap_off(dr, dc):
    colpos = dc // 2 if dc % 2 == 0 else 9 + (dc - 1) // 2
    return 17 * (dr - 1) + colpos

# weight pack column layout: name -> (col0, M, psum partition base)
WBLOCKS = {
    "LR11": (0,   128, 0),
    "LR21": (128, 128, 0),
    "LR22": (256, 128, 0),
    "L20":  (384, 64, 0),
    "L31":  (448, 64, 0),
    "R23":  (512, 64, 64),
    "R33":  (576, 64, 64),
    "U11":  (640, 64, 0),
    "U00":  (704, 64, 0),
    "U01":  (768, 64, 0),
    "U21":  (832, 64, 0),
    "U22":  (896, 64, 0),
}
WPACK_COLS = 960

# (wname, tap, pmin, np, q0, nq); first and last matmul of each
# accumulation group cover the full bank; the L/R-only slots sit
# adjacent so they run concurrently on both column groups.
LR_SLOTS = [
    ("LR11", (1, 1), 0, 8, 0, 8),   # start (full bank, M=128)
    ("LR21", (2, 1), 0, 8, 0, 8),
    ("L20",  (2, 0), 0, 8, 0, 8),   # cols 0-63   } concurrent
    ("R23",  (2, 3), 0, 8, 0, 7),   # cols 64-127 }
    ("L31",  (3, 1), 0, 8, 0, 8),   # cols 0-63   } concurrent
    ("R33",  (3, 3), 0, 8, 0, 7),   # cols 64-127 }
    ("LR22", (2, 2), 0, 8, 0, 8),   # stop (full bank, M=128)
]
U_SLOTS = [
    ("U11",  (1, 1), 0, 8, 0, 8),   # start
    ("U00",  (0, 0), 1, 7, 0, 8),
    ("U01",  (0, 1), 1, 7, 0, 8),
    ("U21",  (2, 1), 0, 8, 0, 8),
    ("U22",  (2, 2), 0, 8, 0, 8),   # stop
]


def _rap(bass, base_ap, nparts, off, dims, part0=0):
    """Raw AP on a tile/tensor: partition pitch from the tile, custom free dims."""
    pitch = base_ap.ap[0][0]
    return bass.AP(
        tensor=base_ap.tensor,
        offset=base_ap.offset + part0 * pitch + off,
        ap=[[pitch, nparts]] + [list(d) for d in dims],
    )


@functools.lru_cache(maxsize=1)
def _build_nc():
    import concourse.bass as bass
    import concourse.bacc as bacc
    import concourse.tile as tile
    from concourse import mybir

    f32 = mybir.dt.float32
    f16 = mybir.dt.float16

    NPAIRS = NTILES // 2
    nc = bacc.Bacc(None)
    # x layout: [half, pair, channel, 2 units contiguous]
    x_d = nc.dram_tensor("x", [2, NPAIRS, C, 2 * UNIT], f16,
                         kind="ExternalInput")
    wp_d = nc.dram_tensor("wpack", [128, WPACK_COLS], f16, kind="ExternalInput")
    lr_d = nc.dram_tensor("lr", [NPAIRS, 128, 2048], f16,
                          kind="ExternalOutput")
    u_d = nc.dram_tensor("u", [NPAIRS, 128, 1024], f16,
                         kind="ExternalOutput")

    with tile.TileContext(nc) as tc:
        with (
            tc.tile_pool(name="singles", bufs=1) as singles,
            tc.tile_pool(name="pslr", bufs=2, space="PSUM") as pslr_pool,
            tc.tile_pool(name="psu", bufs=2, space="PSUM") as psu_pool,
            tc.tile_pool(name="olr", bufs=2) as olr_pool,
            tc.tile_pool(name="ou", bufs=2) as ou_pool,
        ):
            wsb = singles.tile([128, WPACK_COLS], f16)
            nc.gpsimd.dma_start(out=wsb[:], in_=wp_d[:])

            # all-resident input, one SBUF tile per pair of tiles; lower
            # partitions hold units 2p, 2p+1, upper units 16+2p, 16+2p+1
            xg = [singles.tile([128, 2 * UNIT], f16, tag=f"x{p}",
                               name=f"xg{p}") for p in range(NPAIRS)]

            def in_dma(p, h, u0, nu):
                src = bass.AP(
                    tensor=x_d[:].tensor,
                    offset=((h * NPAIRS + p) * C) * (2 * UNIT) + u0 * UNIT,
                    ap=[[2 * UNIT, 64], [1, nu * UNIT]])
                nc.sync.dma_start(
                    out=_rap(bass, xg[p][:], 64, u0 * UNIT,
                             [[1, nu * UNIT]], part0=h * 64),
                    in_=src)

            # pair 0 arrives unit-by-unit so the PE can start early
            for u0 in range(2):
                in_dma(0, 0, u0, 1)
                in_dma(0, 1, u0, 1)
            for p in range(1, NPAIRS):
                in_dma(p, 0, 0, 2)
                in_dma(p, 1, 0, 2)

            def emit(ps, xs, wname, tap, pmin, np_, q0, nq, tg, h, pbase,
                     start, stop):
                c0, m, pscol = WBLOCKS[wname]
                off = _tap_off(*tap) + 34 * pmin
                rhs = _rap(bass, xs[:], 64, tg * UNIT + off,
                           [[IMG, 8], [34, np_], [1, nq]], part0=h * 64)
                out = _rap(bass, ps[:], m, h * 512 + pmin * 8 + q0,
                           [[64, 8], [8, np_], [1, nq]], part0=pbase + pscol)
                lhsT = wsb[h * 64:h * 64 + 64, c0:c0 + m]
                nc.tensor.matmul(out=out, lhsT=lhsT, rhs=rhs,
                                 start=start, stop=stop)

            for pair in range(NPAIRS):
                xs = xg[pair]
                ps_u = psu_pool.tile([128, 1024], f32, tag="psU")
                ps_lr = {}
                for i in (0, 1):
                    ps_lr[i] = pslr_pool.tile([128, 1024], f32, tag="psLR",
                                              name="psLR")
                    n = len(LR_SLOTS)
                    for j, (wname, tap, pmin, np_, q0, nq) in enumerate(LR_SLOTS):
                        for h in (0, 1):
                            emit(ps_lr[i], xs, wname, tap, pmin, np_, q0, nq,
                                 i, h, 0, j == 0, j == n - 1)
                # U taps: even tile of the pair -> psum parts 0-63, odd ->
                # 64-127; with the two row halves that is 4-way concurrency.
                n = len(U_SLOTS)
                for j, (wname, tap, pmin, np_, q0, nq) in enumerate(U_SLOTS):
                    for i in (0, 1):
                        for h in (0, 1):
                            emit(ps_u, xs, wname, tap, pmin, np_, q0, nq,
                                 i, h, 64 * i, j == 0, j == n - 1)

                olr = olr_pool.tile([128, 2048], f16, tag="olr", name="olr")
                ou = ou_pool.tile([128, 1024], f16, tag="ou", name="ou")
                for i in (0, 1):
                    nc.scalar.copy(
                        out=_rap(bass, olr[:], 128, i * 1024, [[1, 1024]]),
                        in_=_rap(bass, ps_lr[i][:], 128, 0, [[1, 1024]]))
                nc.vector.tensor_copy(
                    out=_rap(bass, ou[:], 128, 0, [[1, 1024]]),
                    in_=_rap(bass, ps_u[:], 128, 0, [[1, 1024]]))

                nc.scalar.dma_start(
                    out=bass.AP(tensor=lr_d[:].tensor,
                                offset=pair * 128 * 2048,
                                ap=[[2048, 128], [1, 2048]]),
                    in_=_rap(bass, olr[:], 128, 0, [[1, 2048]]))
                nc.scalar.dma_start(
                    out=bass.AP(tensor=u_d[:].tensor,
                                offset=pair * 128 * 1024,
                                ap=[[1024, 128], [1, 1024]]),
                    in_=_rap(bass, ou[:], 128, 0, [[1, 1024]]))

    nc.finalize()
    return nc


def _host_prep_x(x):
    """x [B, C, 16, 16] f32 -> per-core [2, NTILES//2, C, 2*UNIT] f16."""
    Bn = x.shape[0]
    np_pairs = NTILES // 2
    xp = np.zeros((Bn, C, 18, 18), np.float32)
    xp[:, :, 1:17, 1:17] = x
    xp[:, :, DST_R, DST_C] = xp[:, :, SRC_R, SRC_C]
    a = xp[:, :, 1:, :]                       # rows 1..17
    im = np.concatenate([a[..., 0::2], a[..., 1::2][..., :8]], axis=-1)
    im = np.ascontiguousarray(im.reshape(Bn, C, IMG).astype(np.float16))
    outs = []
    for k in range(NCORES):
        xc = im[k * BC:(k + 1) * BC].reshape(NUNITS, 8, C, IMG)
        xc = xc.transpose(0, 2, 1, 3).reshape(2, np_pairs, 2, C, UNIT)
        outs.append(np.ascontiguousarray(
            xc.transpose(0, 1, 3, 2, 4).reshape(2, np_pairs, C, 2 * UNIT)))
    return outs


def _host_prep_w(w_up, w_left, w_right):
    def wt(w, dr, dc):
        return w[:, :, dr, dc].T.astype(np.float16)  # [c, o]

    wpack = np.zeros((128, WPACK_COLS), np.float16)
    for name, (c0, m, _) in WBLOCKS.items():
        if name.startswith("LR"):
            dr, dc = int(name[2]), int(name[3])
            wpack[0:64, c0:c0 + 64] = wt(w_left, dr, dc)
            wpack[0:64, c0 + 64:c0 + 128] = wt(w_right, dr, dc)
        else:
            dr, dc = int(name[1]), int(name[2])
            w = {"U": w_up, "L": w_left, "R": w_right}[name[0]]
            wpack[0:64, c0:c0 + m] = wt(w, dr, dc)
    wpack[64:128] = wpack[0:64]
    return wpack


def _host_assemble(res, b_up, b_left, b_right):
    """Device outputs -> [B, O, 16, 16] f32 with interleave, bias, mask."""
    np_pairs = NTILES // 2
    Ls, Rs, Us = [], [], []
    for k in range(NCORES):
        lr = res.results[k]["lr"].reshape(np_pairs, 128, 2, 2, 8, 8, 8)
        # (pr, ch, i, h, img, p, q) -> (h, pr, i, img, ch, p, q)
        lr = lr.transpose(3, 0, 2, 4, 1, 5, 6).reshape(BC, 128, 8, 8)
        # u: [pr, i*ch, h, img, p, q] -> (h, pr, i, img, ch, p, q)
        u = res.results[k]["u"].reshape(np_pairs, 2, 64, 2, 8, 8, 8)
        u = u.transpose(3, 0, 1, 4, 2, 5, 6).reshape(BC, 64, 8, 8)
        Ls.append(lr[:, :64]); Rs.append(lr[:, 64:]); Us.append(u)
    L = np.concatenate(Ls, 0).astype(np.float32)
    R = np.concatenate(Rs, 0).astype(np.float32)
    U = np.concatenate(Us, 0).astype(np.float32)
    out = np.zeros((B, O, 16, 16), np.float32)
    out[:, :, 0::2, 0::2] = U + b_up[None, :, None, None]
    out[:, :, 1::2, 0::2] = L + b_left[None, :, None, None]
    out[:, :, 1::2, 1::2] = R + b_right[None, :, None, None]
    out *= OUT_MASK
    return out


LAST_EXEC_NS = None
TRACE = False


def kernel(x, w_up, b_up, w_left, b_left, w_right, b_right):
    global LAST_EXEC_NS
    from concourse.bass_utils import run_bass_kernel_spmd

    x_cores = _host_prep_x(np.asarray(x, np.float32))
    wpack = _host_prep_w(np.asarray(w_up, np.float32),
                         np.asarray(w_left, np.float32),
                         np.asarray(w_right, np.float32))

    nc = _build_nc()
    in_maps = [{"x": x_cores[k], "wpack": wpack} for k in range(NCORES)]
    res = run_bass_kernel_spmd(nc, in_maps, list(range(NCORES)), trace=TRACE)
    LAST_EXEC_NS = res.exec_time_ns
    return _host_assemble(res, np.asarray(b_up, np.float32),
                          np.asarray(b_left, np.float32),
                          np.asarray(b_right, np.float32))
